# revision 7
# baseline (speedup 1.0000x reference)
"""Causal attention block (B=4, S=2048, D=512, f32) on 8 trn2 NeuronCores.

Sharding: 8 cores = 4 batches x 2 query-interleaves. Core (b, h) handles the
eight 128-row query blocks {g : (g//2) % 2 == h} of batch b. Sorted ascending,
their causal K-extents (in 512-wide k-tiles) are [1,1,2,2,3,3,4,4] for BOTH
h=0 and h=1 — a perfectly balanced SPMD program with no wasted k-tiles beyond
the diagonal ones (masked in-tile).

Math per query block (128 rows q, extent E k-tiles):
  L[t]   = Q_blk @ K[:, t*512:(t+1)*512]^T           (PE, 3-pass split matmul)
  Lsb    = -(L + mask)  , rowmax via fused reduce    (DVE tensor_tensor_reduce)
  P      = exp(-Lsb - rowmax), rowsum via accum_out  (ACT)
  PT     = P^T (128x128 chunks)                      (PE transpose + DVE copy)
  O      = (PT^T @ V) * (1/rowsum)                   (PE + DVE)

Precision: QK uses q·k = qr·kr + qcb·kb + qb·kcb where qr=rne12(q) matches the
PE's internal float32r operand rounding exactly (verified on hw), residuals in
bf16 — logits are fp32-exact to ~2e-5. PV runs single-pass float32r (~1e-4).
"""

import sys

sys.path.insert(0, "/opt/trn_rl_repo")

import numpy as np
import ml_dtypes

import concourse.bacc as bacc
import concourse.mybir as mybir
import concourse.tile as tile
from concourse.bass_utils import run_bass_kernel_spmd

B, S, D = 4, 2048, 512
QB = 128          # query rows per block
KT = 512          # k-tile width
NSLOT = 8         # query blocks per core
EXT = [1, 1, 2, 2, 3, 3, 4, 4]   # causal k-tile extent per slot
NEG = -1e30
NCORES = 8

F32 = mybir.dt.float32
F32R = mybir.dt.float32r
BF16 = mybir.dt.bfloat16

QK_SPLIT = True   # 3-pass near-fp32 QK; False = single-pass float32r
STAGE = 5         # debug: 1=dma only, 2=+qk/ttr, 3=+exp, 4=+transpose, 5=full


def _rne12(x: np.ndarray) -> np.ndarray:
    """Round fp32 to 11 explicit mantissa bits, round-to-nearest-even.

    Bit-exact match of the PE's internal float32r operand rounding (measured).
    """
    xi = x.view(np.uint32).astype(np.uint64)
    lsb = (xi >> np.uint64(12)) & np.uint64(1)
    yi = (xi + np.uint64((1 << 11) - 1) + lsb) & ~np.uint64((1 << 12) - 1)
    return (yi & np.uint64(0xFFFFFFFF)).astype(np.uint32).view(np.float32)


def _build_nc():
    nc = bacc.Bacc("TRN2", target_bir_lowering=False, debug=False,
                   num_devices=NCORES)

    qr_d = nc.dram_tensor("qr", [D, NSLOT * QB], F32R, kind="ExternalInput")
    kr_d = nc.dram_tensor("kr", [D, S], F32R, kind="ExternalInput")
    v_d = nc.dram_tensor("v", [S, D], F32R, kind="ExternalInput")
    masks_d = nc.dram_tensor("masks", [3, QB, KT], F32, kind="ExternalInput")
    ident_d = nc.dram_tensor("ident", [QB, QB], F32R, kind="ExternalInput")
    if QK_SPLIT:
        qcb_d = nc.dram_tensor("qcb", [D, NSLOT * QB], BF16, kind="ExternalInput")
        qb_d = nc.dram_tensor("qb", [D, NSLOT * QB], BF16, kind="ExternalInput")
        kcb_d = nc.dram_tensor("kcb", [D, S], BF16, kind="ExternalInput")
        kb_d = nc.dram_tensor("kb", [D, S], BF16, kind="ExternalInput")
    o_d = nc.dram_tensor("o", [NSLOT * QB, D], F32, kind="ExternalOutput")

    NQ = NSLOT * QB   # 1024 query rows per core
    NC_ = 4           # contraction chunks of 128 over D=512
    NKC = S // QB     # 16 k chunks of 128 rows (for V / PT)
    NKT = S // KT     # 4 k tiles

    with tile.TileContext(nc) as tc:
        with (
            tc.tile_pool(name="const", bufs=1) as cpool,
            tc.tile_pool(name="kv", bufs=1) as kvpool,
            tc.tile_pool(name="work", bufs=2) as wpool,
            tc.tile_pool(name="stats", bufs=2) as spool,
            tc.tile_pool(name="qkps", bufs=4, space="PSUM") as qk_ps,
            tc.tile_pool(name="trps", bufs=2, space="PSUM") as tr_ps,
            tc.tile_pool(name="pvps", bufs=2, space="PSUM") as pv_ps,
        ):
            # ---- static / input loads ----
            ident = cpool.tile([QB, QB], F32R, tag="ident")
            nc.sync.dma_start(ident[:], ident_d[:])
            masks = cpool.tile([QB, 3 * KT], F32, tag="masks")
            for j in range(3):
                nc.sync.dma_start(masks[:, j * KT:(j + 1) * KT], masks_d[j])

            qr = kvpool.tile([QB, NC_ * NQ], F32R, tag="qr")
            for c in range(NC_):
                nc.sync.dma_start(qr[:, c * NQ:(c + 1) * NQ],
                                  qr_d[c * QB:(c + 1) * QB, :])
            if QK_SPLIT:
                qcb = kvpool.tile([QB, NC_ * NQ], BF16, tag="qcb")
                qb = kvpool.tile([QB, NC_ * NQ], BF16, tag="qb")
                for c in range(NC_):
                    nc.sync.dma_start(qcb[:, c * NQ:(c + 1) * NQ],
                                      qcb_d[c * QB:(c + 1) * QB, :])
                    nc.sync.dma_start(qb[:, c * NQ:(c + 1) * NQ],
                                      qb_d[c * QB:(c + 1) * QB, :])

            kr = kvpool.tile([QB, NC_ * S], F32R, tag="kr")
            if QK_SPLIT:
                kcb = kvpool.tile([QB, NC_ * S], BF16, tag="kcb")
                kb = kvpool.tile([QB, NC_ * S], BF16, tag="kb")
            v = kvpool.tile([QB, NKC * D], F32R, tag="v")
            # k-tile-major emission so early slots' inputs land first
            for t in range(NKT):
                for c in range(NC_):
                    nc.sync.dma_start(
                        kr[:, c * S + t * KT: c * S + (t + 1) * KT],
                        kr_d[c * QB:(c + 1) * QB, t * KT:(t + 1) * KT])
                    if QK_SPLIT:
                        nc.sync.dma_start(
                            kcb[:, c * S + t * KT: c * S + (t + 1) * KT],
                            kcb_d[c * QB:(c + 1) * QB, t * KT:(t + 1) * KT])
                        nc.sync.dma_start(
                            kb[:, c * S + t * KT: c * S + (t + 1) * KT],
                            kb_d[c * QB:(c + 1) * QB, t * KT:(t + 1) * KT])
                for u in range(KT // QB):
                    ck = t * (KT // QB) + u
                    nc.sync.dma_start(v[:, ck * D:(ck + 1) * D],
                                      v_d[ck * QB:(ck + 1) * QB, :])

            # ---- per-slot attention ----
            for s in range(NSLOT):
                E = EXT[s]
                lsb = wpool.tile([QB, 4 * KT], F32, tag="lsb")
                p = wpool.tile([QB, 4 * KT], F32R, tag="p")
                pt = wpool.tile([QB, 16 * QB], F32R, tag="pt")
                rmax = spool.tile([QB, 1], F32, tag="rmax")
                nm = spool.tile([QB, 1], F32, tag="nm")
                sums = spool.tile([QB, 4], F32, tag="sums")
                ssum = spool.tile([QB, 1], F32, tag="ssum")
                rcp = spool.tile([QB, 1], F32, tag="rcp")

                qsl = slice(s * QB, (s + 1) * QB)
                if STAGE >= 2:
                    for t in range(E):
                        ps = qk_ps.tile([QB, KT], F32, tag="qk")
                        ksl = slice(t * KT, (t + 1) * KT)
                        nmm = 12 if QK_SPLIT else 4
                        i = 0
                        for c in range(NC_):
                            nc.tensor.matmul(
                                ps[:], qr[:, c * NQ:][:, qsl], kr[:, c * S:][:, ksl],
                                start=(i == 0), stop=(i == nmm - 1))
                            i += 1
                        if QK_SPLIT:
                            for c in range(NC_):
                                nc.tensor.matmul(
                                    ps[:], qcb[:, c * NQ:][:, qsl], kb[:, c * S:][:, ksl],
                                    start=False, stop=(i == nmm - 1))
                                i += 1
                            for c in range(NC_):
                                nc.tensor.matmul(
                                    ps[:], qb[:, c * NQ:][:, qsl], kcb[:, c * S:][:, ksl],
                                    start=False, stop=(i == nmm - 1))
                                i += 1
                        # mask + copy to SBUF (lsb = L + M)
                        mj = (s % 2) if t == E - 1 else 2
                        nc.vector.tensor_add(
                            lsb[:, t * KT:(t + 1) * KT],
                            ps[:],
                            masks[:, mj * KT:(mj + 1) * KT])

                    nc.vector.reduce_max(
                        out=rmax[:], in_=lsb[:, :E * KT],
                        axis=mybir.AxisListType.X)
                    nc.vector.tensor_scalar_mul(nm[:], rmax[:], -1.0)

                if STAGE >= 3:
                    for t in range(E):
                        nc.scalar.activation(
                            out=p[:, t * KT:(t + 1) * KT],
                            in_=lsb[:, t * KT:(t + 1) * KT],
                            func=mybir.ActivationFunctionType.Exp,
                            bias=nm[:], scale=1.0,
                            accum_out=sums[:, t:t + 1])

                    nc.vector.tensor_reduce(
                        out=ssum[:], in_=sums[:, :E], axis=mybir.AxisListType.X,
                        op=mybir.AluOpType.add)
                    nc.vector.reciprocal(rcp[:], ssum[:])

                if STAGE >= 4:
                    for t in range(E):
                        trp = tr_ps.tile([QB, KT], F32R, tag="tr")
                        for j in range(4):
                            nc.tensor.transpose(
                                trp[:, j * QB:(j + 1) * QB],
                                p[:, t * KT + j * QB: t * KT + (j + 1) * QB],
                                ident[:])
                        nc.vector.tensor_copy(pt[:, t * KT:(t + 1) * KT], trp[:])

                osb = wpool.tile([QB, D], F32, tag="o")
                if STAGE >= 5:
                    pvp = pv_ps.tile([QB, D], F32, tag="pv")
                    for m in range(4 * E):
                        nc.tensor.matmul(
                            pvp[:], pt[:, m * QB:(m + 1) * QB], v[:, m * D:(m + 1) * D],
                            start=(m == 0), stop=(m == 4 * E - 1))
                    nc.vector.tensor_scalar_mul(osb[:], pvp[:], rcp[:])
                else:
                    nc.vector.memset(osb[:], 0.0)
                nc.sync.dma_start(o_d[s * QB:(s + 1) * QB, :], osb[:])

    nc.compile()
    return nc


_NC_CACHE = {}


def _get_nc():
    if "nc" not in _NC_CACHE:
        _NC_CACHE["nc"] = _build_nc()
    return _NC_CACHE["nc"]


def _core_blocks(h):
    return [g for g in range(16) if (g // 2) % 2 == h]


def _shard_inputs(keys, queries, values):
    bf16 = ml_dtypes.bfloat16
    ident = np.eye(QB, dtype=np.float32)
    in_maps = []
    # masks per core type h: parity j -> additive mask for m = 2h + j
    mrow = np.arange(QB, dtype=np.int64)[:, None]
    mcol = np.arange(KT, dtype=np.int64)[None, :]
    masks_h = []
    for h in range(2):
        mk = np.zeros((3, QB, KT), np.float32)
        for j in range(2):
            m = 2 * h + j
            mk[j] = np.where(mcol <= m * QB + mrow, 0.0, NEG).astype(np.float32)
        masks_h.append(mk)

    for b in range(B):
        k = np.ascontiguousarray(keys[b])        # [S, D]
        vv = np.ascontiguousarray(values[b])     # [S, D]
        krT = np.ascontiguousarray(_rne12(k).T)  # [D, S]
        if QK_SPLIT:
            kcbT = np.ascontiguousarray((k - _rne12(k)).T.astype(bf16))
            kbT = np.ascontiguousarray(k.T.astype(bf16))
        for h in range(2):
            G = _core_blocks(h)
            q = np.concatenate(
                [queries[b, g * QB:(g + 1) * QB, :] for g in G], axis=0)  # [1024, D]
            qrT = np.ascontiguousarray(_rne12(q).T)
            im = {
                "qr": qrT, "kr": krT, "v": vv,
                "masks": masks_h[h], "ident": ident,
            }
            if QK_SPLIT:
                im["qcb"] = np.ascontiguousarray((q - _rne12(q)).T.astype(bf16))
                im["qb"] = np.ascontiguousarray(q.T.astype(bf16))
                im["kcb"] = kcbT
                im["kb"] = kbT
            in_maps.append(im)
    return in_maps


def kernel(keys, queries, values, original, _trace=False, _trace_cores=None):
    keys = np.asarray(keys, dtype=np.float32)
    queries = np.asarray(queries, dtype=np.float32)
    values = np.asarray(values, dtype=np.float32)
    original = np.asarray(original, dtype=np.float32)

    nc = _get_nc()
    in_maps = _shard_inputs(keys, queries, values)
    res = run_bass_kernel_spmd(
        nc, in_maps, list(range(NCORES)),
        trace=_trace, trace_cores=_trace_cores)

    read = np.empty((B, S, D), np.float32)
    for b in range(B):
        for h in range(2):
            o = res.results[b * 2 + h]["o"]
            for s, g in enumerate(_core_blocks(h)):
                read[b, g * QB:(g + 1) * QB, :] = o[s * QB:(s + 1) * QB, :]

    out = np.concatenate([original, read], axis=-1)
    if _trace:
        return out, res
    return out
